# revision 39
# baseline (speedup 1.0000x reference)
"""Trainium2 Bass kernel for the Diversity4 loss.

Math (per sample b, models m=0..3, classes c=0..999):
    p_m = softmax(x_m / T);  v_m = (p_m - mean(p_m)) / ||p_m - mean(p_m)||
    d_b = sum_{j<k} v_j . v_k ;  answer = mean(SCALE * d_b)

Simplifications used:
  * Softmax normalization cancels:  v_m = (e_m - mean(e_m)) / ||e_m - mean(e_m)||
    with e_m = exp(x_m / T).
  * Pairwise-dot sum: d_b = 0.5 * (||s||^2 - sum_m ||v_m||^2),  s = sum_m v_m.
  * Each v_m is centered, so s is centered and ||s||^2 = C * var(s); variance
    is shift-invariant, so with w = sum_m rnorm_m * e_m we get
    ||s||^2 = C * var(w) -- no mean correction needed.
  * sum_m ||v_m||^2 (the tau correction) is computed explicitly rather than
    assumed == 4, so rsqrt bias cancels to first order.  Only its GLOBAL sum
    matters:  sum_b d_b = C * (sum_b var(w_b) - sum_{b,m} var_m * rnorm_m^2),
    so tau reduces to cheap elementwise sums -- no extra matmul.
  * rnorm = exp(-0.5 * ln(C * var)): ln and exp live in one ACT table set
    (natural_log_exp_and_others), so the kernel never switches ACT tables
    (a switch costs ~2.7us).  Copy/Square are set-resident fillers.
  * The w-sum over models runs on the TensorEngine in float32r (~12-bit
    mantissa, 1 cycle/col vs 4 for fp32).  Rounding noise is zero-mean and
    contributes <0.1% to the final mean; the tau correction uses the same
    rounded rnorm so normalization stays self-consistent.

Layout: 8 cores x 2048 samples.  Per core, 16 groups of 512 rows of the
host-interleaved xall (row 4*s+m = model m of sample s).  Each group is one
[128, 4, 1000] SBUF tile: partition p = 4*s' + m (s' = s mod 32), free dim =
(tile j, class c).  Per group:
  DMA(1): 2 MB contiguous load.
  ACT: exp -> E (f32r) once over [128, 4000]; ln/exp/square for rnorm [128,4];
       4x lhsT = Z_j * rnorm ([128,128] f32r, Z_j inline block-diag masks).
  DVE: one batched bn_stats [128,8,500] + 4 bn_aggr -> var(e); tau partial
       sums; one batched bn_stats over the W PSUM tile -> var(w); accumulate.
  PE:  8 accumulating f32r matmuls stack w for 4 tiles into one [128, 1000]
       PSUM tile (lhsT zero outside col band 32j..32j+32).
Final: PE ones-matmul folds [128,2] (sum var_w | sum tau) across partitions.
Host: answer = SCALE * 0.5 * C * (sum_w - sum_tau) / B.
"""

import contextlib
import sys

import numpy as np

for _p in ("/opt/trn_rl_repo",):
    if _p not in sys.path:
        sys.path.insert(0, _p)

import concourse.bacc as bacc
import concourse.tile as tile
from concourse import mybir
from concourse import bass_utils

B, C = 16384, 1000
N_CORES = 8
B_LOC = B // N_CORES  # 2048
SPT = 32  # samples per tile (x 4 models = 128 partitions)
GROUP = 4  # tiles per PSUM stack / load group
N_GROUPS = B_LOC // (SPT * GROUP)  # 16
T = 20.0
SCALE = 0.3

F32 = mybir.dt.float32
F32R = mybir.dt.float32r
AF = mybir.ActivationFunctionType


def _build_nc():
    nc = bacc.Bacc("TRN2")
    # xall rows are interleaved (sample-major): row 4*s + m = model m of sample s
    xall = nc.dram_tensor("xall", [B_LOC * 4, C], F32, kind="ExternalInput")
    out = nc.dram_tensor("partial", [1, 1], F32, kind="ExternalOutput")

    # Z[j][p, 32*j + p//4] = 1, zero elsewhere: per-tile block-diagonal
    # scatter masks; lhsT_j = Z_j * rnorm stacks tile j's samples into
    # output partitions 32j..32j+32 of the shared accumulating PSUM tile.
    z_np = np.zeros((GROUP, 128, 128), dtype=np.float32)
    for j in range(GROUP):
        z_np[j, np.arange(128), SPT * j + np.arange(128) // 4] = 1.0
    z_dram = nc.inline_tensor(np.ascontiguousarray(z_np), name="zmasks")

    with tile.TileContext(nc) as tc:
        with (
            tc.tile_pool(name="singles", bufs=1) as singles,
            tc.tile_pool(name="xpool", bufs=4) as xpool,
            tc.tile_pool(name="epool", bufs=3) as epool,
            tc.tile_pool(name="small", bufs=6) as small,
            tc.tile_pool(name="lhspool", bufs=8) as lhspool,
            tc.tile_pool(name="wpsum", bufs=2, space="PSUM") as wpsum,
        ):
            zmasks = singles.tile([128, GROUP, 128], F32)
            nc.gpsimd.dma_start(
                out=zmasks, in_=z_dram.rearrange("j p q -> p j q")
            )
            ones = singles.tile([128, 1], F32)
            nc.vector.memset(ones, 1.0)
            warm = singles.tile([128, 1], F32)
            nc.scalar.activation(warm, ones, AF.Exp)
            # accs[:,0] accumulates var(w); accs[:,1] accumulates tau
            accs = singles.tile([128, 2], F32)
            nc.vector.memset(accs, 0.0)

            for g in range(N_GROUPS):
                r0 = g * GROUP * 128  # row offset into xall
                # boost the tail group's scheduler priority so its chain is
                # never queued behind lingering earlier-group work
                prio = (
                    tc.high_priority()
                    if g == N_GROUPS - 1
                    else contextlib.nullcontext()
                )
                prio.__enter__()
                X = xpool.tile([128, GROUP, C], F32, tag="X")
                for j in range(GROUP):
                    nc.sync.dma_start(
                        out=X[:, j, :],
                        in_=xall[r0 + 128 * j : r0 + 128 * (j + 1), :],
                    )
                fine = g == N_GROUPS - 1
                E = epool.tile([128, GROUP, C], F32R, tag="E")
                if fine:
                    # last group: per-tile exp so each tile's chain drains
                    # without waiting for the whole 4000-wide activation
                    for j in range(GROUP):
                        nc.scalar.activation(
                            E[:, j, :], X[:, j, :], AF.Exp, scale=1.0 / T
                        )
                else:
                    nc.scalar.activation(E, X, AF.Exp, scale=1.0 / T)
                Ef = E.bitcast(F32)
                stats = small.tile([128, 2 * GROUP, 6], F32, tag="stats")
                Ev = Ef.rearrange("p j (h x) -> p (j h) x", h=2)
                for h in range(2 * GROUP):
                    nc.vector.bn_stats(stats[:, h, :], Ev[:, h, :])
                mv = small.tile([128, GROUP, 2], F32, tag="mv")
                for j in range(GROUP):
                    nc.vector.bn_aggr(mv[:, j, :], stats[:, 2 * j : 2 * j + 2, :])
                varv = mv[:, :, 1]  # [128, GROUP] strided view
                # rnorm = exp(-0.5 * ln(C * var)); ln+exp share one table set
                L = small.tile([128, GROUP], F32, tag="L")
                rn = small.tile([128, GROUP], F32R, tag="rn")
                if fine:
                    for j in range(GROUP):
                        nc.scalar.activation(
                            L[:, j : j + 1], varv[:, j : j + 1], AF.Ln,
                            scale=float(C),
                        )
                        nc.scalar.activation(
                            rn[:, j : j + 1], L[:, j : j + 1], AF.Exp,
                            scale=-0.5,
                        )
                else:
                    nc.scalar.activation(L, varv, AF.Ln, scale=float(C))
                    nc.scalar.activation(rn, L, AF.Exp, scale=-0.5)
                rnf = rn.bitcast(F32)
                # tau partials: t = var * rnorm^2; only sum_{p,j} t is needed
                tg = small.tile([128, GROUP], F32, tag="tg")
                nc.vector.tensor_mul(tg, rnf, rnf)
                nc.vector.tensor_mul(tg, tg, varv)
                tsum = small.tile([128, 1], F32, tag="tsum")
                nc.vector.tensor_reduce(
                    tsum, tg, axis=mybir.AxisListType.X, op=mybir.AluOpType.add
                )
                nc.vector.tensor_add(accs[:, 1:2], accs[:, 1:2], tsum)

                # rm = rnorm * mean(e): weights for the w-mean (wbar) matmul
                meanv = mv[:, :, 0]
                rm = small.tile([128, GROUP], F32, tag="rm")
                nc.vector.tensor_mul(rm, rnf, meanv)
                W = wpsum.tile([128, 1024], F32, tag="W")
                Wb = wpsum.tile([128, 8], F32, tag="Wb")
                for j in range(GROUP):
                    lhsT = lhspool.tile([128, 128], F32R, tag="lhsT")
                    with nc.allow_low_precision(reason="f32r lhsT for PE"):
                        nc.gpsimd.tensor_scalar_mul(
                            lhsT, zmasks[:, j, :], rnf[:, j : j + 1]
                        )
                    lhsT2 = lhspool.tile([128, 128], F32, tag="lhsT2")
                    nc.gpsimd.tensor_scalar_mul(
                        lhsT2, zmasks[:, j, :], rm[:, j : j + 1]
                    )
                    first, last = (j == 0), (j == GROUP - 1)
                    nc.tensor.matmul(
                        W[:, 0:512],
                        lhsT,
                        E[:, j, 0:512],
                        start=first,
                        stop=last,
                        skip_group_check=True,
                    )
                    nc.tensor.matmul(
                        W[:, 512:1000],
                        lhsT,
                        E[:, j, 512:1000],
                        start=first,
                        stop=last,
                        skip_group_check=True,
                    )
                    nc.tensor.matmul(
                        Wb[:, 0:1],
                        lhsT2,
                        ones,
                        start=first,
                        stop=last,
                        skip_group_check=True,
                    )
                # C*var(w) = sum_c (w - wbar)^2 via ACT Square+accum (both
                # chunks), freeing the DVE of the W-variance stream.
                wbneg = small.tile([128, 1], F32, tag="wbneg")
                nc.scalar.activation(wbneg, Wb[:, 0:1], AF.Copy, scale=-1.0)
                sqscr = small.tile([128, 1024], F32, tag="sqscr", bufs=2)
                q = small.tile([128, 2], F32, tag="q")
                nc.scalar.activation(
                    sqscr[:, 0:512],
                    W[:, 0:512],
                    AF.Square,
                    bias=wbneg,
                    accum_out=q[:, 0:1],
                )
                nc.scalar.activation(
                    sqscr[:, 512:1000],
                    W[:, 512:1000],
                    AF.Square,
                    bias=wbneg,
                    accum_out=q[:, 1:2],
                )
                nc.vector.tensor_add(accs[:, 0:1], accs[:, 0:1], q[:, 0:1])
                nc.vector.tensor_add(accs[:, 0:1], accs[:, 0:1], q[:, 1:2])

            # d-accumulator = accs[:,0] - C*accs[:,1], folded before the
            # cross-partition reduce so the big tau/var magnitudes cancel in
            # f32 while still small.
            dacc = singles.tile([128, 1], F32)
            nc.vector.tensor_scalar(
                dacc,
                accs[:, 1:2],
                float(-C),
                None,
                op0=mybir.AluOpType.mult,
            )
            nc.vector.tensor_add(dacc, dacc, accs[:, 0:1])
            fin = wpsum.tile([1, 1], F32, tag="fin", bufs=1)
            nc.tensor.matmul(fin, dacc, ones, start=True, stop=True)
            res = singles.tile([1, 1], F32)
            nc.scalar.copy(res, fin)
            nc.sync.dma_start(out=out[:, :], in_=res)
    _strip_redundant_dma_waits(nc)
    # Force the ACT table chooser onto the one set that serves every function
    # this kernel uses (exp, ln, square, copy, identity).  The default greedy
    # chooser picks exp_and_others for Exp and natural_log for Ln, inserting
    # a ~2.7us table swap per activation pair, ~86us/core of pure overhead.
    _orig_tables = bacc.get_activation_tables

    def _only_shared(arch):
        tabs = _orig_tables(arch)
        return {
            name: (fns if name == "natural_log_exp_and_others" else set())
            for name, fns in tabs.items()
        }

    bacc.get_activation_tables = _only_shared
    try:
        nc.finalize()
    finally:
        bacc.get_activation_tables = _orig_tables
    return nc


def _strip_redundant_dma_waits(nc):
    """Drop same-queue WAW waits on load DMAs.

    The HWDGE DMA pseudo-instruction accepts a single sync wait, but Tile
    emits two once SBUF slots recycle: [engine-sem release by the slot's
    reader, own-queue WAW vs the slot's previous writer].  The WAW wait is
    redundant when the kept reader wait transitively orders the new write
    after the old one.  Only remove a wait that targets the exact semaphore
    this DMA updates (same queue) while an engine-semaphore wait remains;
    anything else is left for Bacc's event-semaphore splitting.
    """
    removed = 0
    for inst in nc.inst_map.values():
        if type(inst).__name__ != "InstDMACopy":
            continue
        si = getattr(inst, "sync_info", None)
        if si is None:
            continue
        waits = list(si.on_wait or [])
        if len(waits) <= 1:
            continue
        upd_names = {u.ant_name for u in (si.on_update or [])}
        drop = [w for w in waits if w.ant_name in upd_names]
        keep = [w for w in waits if w.ant_name not in upd_names]
        if len(drop) != 1 or not keep:
            continue
        if any(k.ant_name.startswith("DMA") for k in keep):
            continue
        si.on_wait = keep
        inst.sync_info = si
        removed += 1
    return removed


_NC_CACHE = {}


def _get_nc():
    if "nc" not in _NC_CACHE:
        _NC_CACHE["nc"] = _build_nc()
    return _NC_CACHE["nc"]


def run_on_cores(arrays, trace=False):
    """arrays: list of 4 full [B, C] f32 arrays. Returns (partials, results)."""
    nc = _get_nc()
    # Interleave models per sample: xall[4*s + m, c] = arrays[m][s, c]
    xall = np.ascontiguousarray(
        np.stack(arrays, axis=1).reshape(B * 4, C).astype(np.float32, copy=False)
    )
    in_maps = []
    for k in range(N_CORES):
        in_maps.append({"xall": xall[k * B_LOC * 4 : (k + 1) * B_LOC * 4]})
    res = bass_utils.run_bass_kernel_spmd(
        nc, in_maps, core_ids=list(range(N_CORES)), trace=trace
    )
    # per-core partial = sum_b d_b (tau already folded on device)
    partials = [float(r["partial"][0, 0]) for r in res.results]
    return partials, res


def kernel(outputs1, outputs2, outputs3, outputs4, targets=None):
    arrays = [
        np.ascontiguousarray(np.asarray(a, dtype=np.float32))
        for a in (outputs1, outputs2, outputs3, outputs4)
    ]
    partials, _ = run_on_cores(arrays, trace=False)
    total = float(np.sum(np.asarray(partials, dtype=np.float64)))
    ans = SCALE * 0.5 * total / B
    return np.array(ans, dtype=np.float32)


# revision 40
# speedup vs baseline: 1.0669x; 1.0669x over previous
"""Trainium2 Bass kernel for the Diversity4 loss.

Math (per sample b, models m=0..3, classes c=0..999):
    p_m = softmax(x_m / T);  v_m = (p_m - mean(p_m)) / ||p_m - mean(p_m)||
    d_b = sum_{j<k} v_j . v_k ;  answer = mean(SCALE * d_b)

Simplifications used:
  * Softmax normalization cancels:  v_m = (e_m - mean(e_m)) / ||e_m - mean(e_m)||
    with e_m = exp(x_m / T).
  * Pairwise-dot sum: d_b = 0.5 * (||s||^2 - sum_m ||v_m||^2),  s = sum_m v_m.
  * Each v_m is centered, so s is centered and ||s||^2 = C * var(s); variance
    is shift-invariant, so with w = sum_m rnorm_m * e_m we get
    ||s||^2 = C * var(w) -- no mean correction needed.
  * sum_m ||v_m||^2 (the tau correction) is computed explicitly rather than
    assumed == 4, so rsqrt bias cancels to first order.  Only its GLOBAL sum
    matters:  sum_b d_b = C * (sum_b var(w_b) - sum_{b,m} var_m * rnorm_m^2),
    so tau reduces to cheap elementwise sums -- no extra matmul.
  * rnorm = exp(-0.5 * ln(C * var)): ln and exp live in one ACT table set
    (natural_log_exp_and_others), so the kernel never switches ACT tables
    (a switch costs ~2.7us).  Copy/Square are set-resident fillers.
  * The w-sum over models runs on the TensorEngine in float32r (~12-bit
    mantissa, 1 cycle/col vs 4 for fp32).  Rounding noise is zero-mean and
    contributes <0.1% to the final mean; the tau correction uses the same
    rounded rnorm so normalization stays self-consistent.

Layout: 8 cores x 2048 samples.  Per core, 16 groups of 512 rows of the
host-interleaved xall (row 4*s+m = model m of sample s).  Each group is one
[128, 4, 1000] SBUF tile: partition p = 4*s' + m (s' = s mod 32), free dim =
(tile j, class c).  Per group:
  DMA(1): 2 MB contiguous load.
  ACT: exp -> E (f32r) once over [128, 4000]; ln/exp/square for rnorm [128,4];
       4x lhsT = Z_j * rnorm ([128,128] f32r, Z_j inline block-diag masks).
  DVE: one batched bn_stats [128,8,500] + 4 bn_aggr -> var(e); tau partial
       sums; one batched bn_stats over the W PSUM tile -> var(w); accumulate.
  PE:  8 accumulating f32r matmuls stack w for 4 tiles into one [128, 1000]
       PSUM tile (lhsT zero outside col band 32j..32j+32).
Final: PE ones-matmul folds [128,2] (sum var_w | sum tau) across partitions.
Host: answer = SCALE * 0.5 * C * (sum_w - sum_tau) / B.
"""

import contextlib
import sys

import numpy as np

for _p in ("/opt/trn_rl_repo",):
    if _p not in sys.path:
        sys.path.insert(0, _p)

import concourse.bacc as bacc
import concourse.tile as tile
from concourse import mybir
from concourse import bass_utils

B, C = 16384, 1000
N_CORES = 8
B_LOC = B // N_CORES  # 2048
SPT = 32  # samples per tile (x 4 models = 128 partitions)
GROUP = 4  # tiles per PSUM stack / load group
N_GROUPS = B_LOC // (SPT * GROUP)  # 16
T = 20.0
SCALE = 0.3

F32 = mybir.dt.float32
F32R = mybir.dt.float32r
AF = mybir.ActivationFunctionType


def _build_nc():
    nc = bacc.Bacc("TRN2")
    # xall rows are interleaved (sample-major): row 4*s + m = model m of sample s
    xall = nc.dram_tensor("xall", [B_LOC * 4, C], F32, kind="ExternalInput")
    out = nc.dram_tensor("partial", [1, 1], F32, kind="ExternalOutput")

    # Z[j][p, 32*j + p//4] = 1, zero elsewhere: per-tile block-diagonal
    # scatter masks; lhsT_j = Z_j * rnorm stacks tile j's samples into
    # output partitions 32j..32j+32 of the shared accumulating PSUM tile.
    z_np = np.zeros((GROUP, 128, 128), dtype=np.float32)
    for j in range(GROUP):
        z_np[j, np.arange(128), SPT * j + np.arange(128) // 4] = 1.0
    z_dram = nc.inline_tensor(np.ascontiguousarray(z_np), name="zmasks")

    with tile.TileContext(nc) as tc:
        with (
            tc.tile_pool(name="singles", bufs=1) as singles,
            tc.tile_pool(name="xpool", bufs=4) as xpool,
            tc.tile_pool(name="epool", bufs=3) as epool,
            tc.tile_pool(name="small", bufs=6) as small,
            tc.tile_pool(name="lhspool", bufs=8) as lhspool,
            tc.tile_pool(name="wpsum", bufs=2, space="PSUM") as wpsum,
        ):
            zmasks = singles.tile([128, GROUP, 128], F32)
            nc.gpsimd.dma_start(
                out=zmasks, in_=z_dram.rearrange("j p q -> p j q")
            )
            ones = singles.tile([128, 1], F32)
            nc.vector.memset(ones, 1.0)
            warm = singles.tile([128, 1], F32)
            nc.scalar.activation(warm, ones, AF.Exp)
            # accs[:,0] accumulates var(w); accs[:,1] accumulates tau
            accs = singles.tile([128, 2], F32)
            nc.vector.memset(accs, 0.0)

            for g in range(N_GROUPS):
                r0 = g * GROUP * 128  # row offset into xall
                # boost the tail group's scheduler priority so its chain is
                # never queued behind lingering earlier-group work
                X = xpool.tile([128, GROUP, C], F32, tag="X")
                for j in range(GROUP):
                    nc.sync.dma_start(
                        out=X[:, j, :],
                        in_=xall[r0 + 128 * j : r0 + 128 * (j + 1), :],
                    )
                prio = (
                    tc.high_priority()
                    if g == N_GROUPS - 1
                    else contextlib.nullcontext()
                )
                prio.__enter__()
                fine = g == N_GROUPS - 1
                E = epool.tile([128, GROUP, C], F32R, tag="E")
                if fine:
                    # last group: per-tile exp so each tile's chain drains
                    # without waiting for the whole 4000-wide activation
                    for j in range(GROUP):
                        nc.scalar.activation(
                            E[:, j, :], X[:, j, :], AF.Exp, scale=1.0 / T
                        )
                else:
                    nc.scalar.activation(E, X, AF.Exp, scale=1.0 / T)
                Ef = E.bitcast(F32)
                stats = small.tile([128, 2 * GROUP, 6], F32, tag="stats")
                Ev = Ef.rearrange("p j (h x) -> p (j h) x", h=2)
                for h in range(2 * GROUP):
                    nc.vector.bn_stats(stats[:, h, :], Ev[:, h, :])
                mv = small.tile([128, GROUP, 2], F32, tag="mv")
                for j in range(GROUP):
                    nc.vector.bn_aggr(mv[:, j, :], stats[:, 2 * j : 2 * j + 2, :])
                varv = mv[:, :, 1]  # [128, GROUP] strided view
                # rnorm = exp(-0.5 * ln(C * var)); ln+exp share one table set
                L = small.tile([128, GROUP], F32, tag="L")
                rn = small.tile([128, GROUP], F32R, tag="rn")
                if fine:
                    for j in range(GROUP):
                        nc.scalar.activation(
                            L[:, j : j + 1], varv[:, j : j + 1], AF.Ln,
                            scale=float(C),
                        )
                        nc.scalar.activation(
                            rn[:, j : j + 1], L[:, j : j + 1], AF.Exp,
                            scale=-0.5,
                        )
                else:
                    nc.scalar.activation(L, varv, AF.Ln, scale=float(C))
                    nc.scalar.activation(rn, L, AF.Exp, scale=-0.5)
                rnf = rn.bitcast(F32)
                # tau partials: t = var * rnorm^2; only sum_{p,j} t is needed
                tg = small.tile([128, GROUP], F32, tag="tg")
                nc.vector.tensor_mul(tg, rnf, rnf)
                nc.vector.tensor_mul(tg, tg, varv)
                tsum = small.tile([128, 1], F32, tag="tsum")
                nc.vector.tensor_reduce(
                    tsum, tg, axis=mybir.AxisListType.X, op=mybir.AluOpType.add
                )
                nc.vector.tensor_add(accs[:, 1:2], accs[:, 1:2], tsum)

                # rm = rnorm * mean(e): weights for the w-mean (wbar) matmul
                meanv = mv[:, :, 0]
                rm = small.tile([128, GROUP], F32, tag="rm")
                nc.vector.tensor_mul(rm, rnf, meanv)
                W = wpsum.tile([128, 1024], F32, tag="W")
                Wb = wpsum.tile([128, 8], F32, tag="Wb")
                for j in range(GROUP):
                    lhsT = lhspool.tile([128, 128], F32R, tag="lhsT")
                    with nc.allow_low_precision(reason="f32r lhsT for PE"):
                        nc.gpsimd.tensor_scalar_mul(
                            lhsT, zmasks[:, j, :], rnf[:, j : j + 1]
                        )
                    lhsT2 = lhspool.tile([128, 128], F32, tag="lhsT2")
                    nc.gpsimd.tensor_scalar_mul(
                        lhsT2, zmasks[:, j, :], rm[:, j : j + 1]
                    )
                    first, last = (j == 0), (j == GROUP - 1)
                    nc.tensor.matmul(
                        W[:, 0:512],
                        lhsT,
                        E[:, j, 0:512],
                        start=first,
                        stop=last,
                        skip_group_check=True,
                    )
                    nc.tensor.matmul(
                        W[:, 512:1000],
                        lhsT,
                        E[:, j, 512:1000],
                        start=first,
                        stop=last,
                        skip_group_check=True,
                    )
                    nc.tensor.matmul(
                        Wb[:, 0:1],
                        lhsT2,
                        ones,
                        start=first,
                        stop=last,
                        skip_group_check=True,
                    )
                # C*var(w) = sum_c (w - wbar)^2 via ACT Square+accum (both
                # chunks), freeing the DVE of the W-variance stream.
                wbneg = small.tile([128, 1], F32, tag="wbneg")
                nc.scalar.activation(wbneg, Wb[:, 0:1], AF.Copy, scale=-1.0)
                sqscr = small.tile([128, 1024], F32, tag="sqscr", bufs=2)
                q = small.tile([128, 2], F32, tag="q")
                nc.scalar.activation(
                    sqscr[:, 0:512],
                    W[:, 0:512],
                    AF.Square,
                    bias=wbneg,
                    accum_out=q[:, 0:1],
                )
                nc.scalar.activation(
                    sqscr[:, 512:1000],
                    W[:, 512:1000],
                    AF.Square,
                    bias=wbneg,
                    accum_out=q[:, 1:2],
                )
                nc.vector.tensor_add(accs[:, 0:1], accs[:, 0:1], q[:, 0:1])
                nc.vector.tensor_add(accs[:, 0:1], accs[:, 0:1], q[:, 1:2])

            # d-accumulator = accs[:,0] - C*accs[:,1], folded before the
            # cross-partition reduce so the big tau/var magnitudes cancel in
            # f32 while still small.
            dacc = singles.tile([128, 1], F32)
            nc.vector.tensor_scalar(
                dacc,
                accs[:, 1:2],
                float(-C),
                None,
                op0=mybir.AluOpType.mult,
            )
            nc.vector.tensor_add(dacc, dacc, accs[:, 0:1])
            fin = wpsum.tile([1, 1], F32, tag="fin", bufs=1)
            nc.tensor.matmul(fin, dacc, ones, start=True, stop=True)
            res = singles.tile([1, 1], F32)
            nc.scalar.copy(res, fin)
            nc.sync.dma_start(out=out[:, :], in_=res)
    _strip_redundant_dma_waits(nc)
    # Force the ACT table chooser onto the one set that serves every function
    # this kernel uses (exp, ln, square, copy, identity).  The default greedy
    # chooser picks exp_and_others for Exp and natural_log for Ln, inserting
    # a ~2.7us table swap per activation pair, ~86us/core of pure overhead.
    _orig_tables = bacc.get_activation_tables

    def _only_shared(arch):
        tabs = _orig_tables(arch)
        return {
            name: (fns if name == "natural_log_exp_and_others" else set())
            for name, fns in tabs.items()
        }

    bacc.get_activation_tables = _only_shared
    try:
        nc.finalize()
    finally:
        bacc.get_activation_tables = _orig_tables
    return nc


def _strip_redundant_dma_waits(nc):
    """Drop same-queue WAW waits on load DMAs.

    The HWDGE DMA pseudo-instruction accepts a single sync wait, but Tile
    emits two once SBUF slots recycle: [engine-sem release by the slot's
    reader, own-queue WAW vs the slot's previous writer].  The WAW wait is
    redundant when the kept reader wait transitively orders the new write
    after the old one.  Only remove a wait that targets the exact semaphore
    this DMA updates (same queue) while an engine-semaphore wait remains;
    anything else is left for Bacc's event-semaphore splitting.
    """
    removed = 0
    for inst in nc.inst_map.values():
        if type(inst).__name__ != "InstDMACopy":
            continue
        si = getattr(inst, "sync_info", None)
        if si is None:
            continue
        waits = list(si.on_wait or [])
        if len(waits) <= 1:
            continue
        upd_names = {u.ant_name for u in (si.on_update or [])}
        drop = [w for w in waits if w.ant_name in upd_names]
        keep = [w for w in waits if w.ant_name not in upd_names]
        if len(drop) != 1 or not keep:
            continue
        if any(k.ant_name.startswith("DMA") for k in keep):
            continue
        si.on_wait = keep
        inst.sync_info = si
        removed += 1
    return removed


_NC_CACHE = {}


def _get_nc():
    if "nc" not in _NC_CACHE:
        _NC_CACHE["nc"] = _build_nc()
    return _NC_CACHE["nc"]


def run_on_cores(arrays, trace=False):
    """arrays: list of 4 full [B, C] f32 arrays. Returns (partials, results)."""
    nc = _get_nc()
    # Interleave models per sample: xall[4*s + m, c] = arrays[m][s, c]
    xall = np.ascontiguousarray(
        np.stack(arrays, axis=1).reshape(B * 4, C).astype(np.float32, copy=False)
    )
    in_maps = []
    for k in range(N_CORES):
        in_maps.append({"xall": xall[k * B_LOC * 4 : (k + 1) * B_LOC * 4]})
    res = bass_utils.run_bass_kernel_spmd(
        nc, in_maps, core_ids=list(range(N_CORES)), trace=trace
    )
    # per-core partial = sum_b d_b (tau already folded on device)
    partials = [float(r["partial"][0, 0]) for r in res.results]
    return partials, res


def kernel(outputs1, outputs2, outputs3, outputs4, targets=None):
    arrays = [
        np.ascontiguousarray(np.asarray(a, dtype=np.float32))
        for a in (outputs1, outputs2, outputs3, outputs4)
    ]
    partials, _ = run_on_cores(arrays, trace=False)
    total = float(np.sum(np.asarray(partials, dtype=np.float64)))
    ans = SCALE * 0.5 * total / B
    return np.array(ans, dtype=np.float32)


# revision 41
# speedup vs baseline: 1.1166x; 1.0466x over previous
"""Trainium2 Bass kernel for the Diversity4 loss.

Math (per sample b, models m=0..3, classes c=0..999):
    p_m = softmax(x_m / T);  v_m = (p_m - mean(p_m)) / ||p_m - mean(p_m)||
    d_b = sum_{j<k} v_j . v_k ;  answer = mean(SCALE * d_b)

Simplifications used:
  * Softmax normalization cancels:  v_m = (e_m - mean(e_m)) / ||e_m - mean(e_m)||
    with e_m = exp(x_m / T).
  * Pairwise-dot sum: d_b = 0.5 * (||s||^2 - sum_m ||v_m||^2),  s = sum_m v_m.
  * Each v_m is centered, so s is centered and ||s||^2 = C * var(s); variance
    is shift-invariant, so with w = sum_m rnorm_m * e_m we get
    ||s||^2 = C * var(w) -- no mean correction needed.
  * sum_m ||v_m||^2 (the tau correction) is computed explicitly rather than
    assumed == 4, so rsqrt bias cancels to first order.  Only its GLOBAL sum
    matters:  sum_b d_b = C * (sum_b var(w_b) - sum_{b,m} var_m * rnorm_m^2),
    so tau reduces to cheap elementwise sums -- no extra matmul.
  * rnorm = exp(-0.5 * ln(C * var)): ln and exp live in one ACT table set
    (natural_log_exp_and_others), so the kernel never switches ACT tables
    (a switch costs ~2.7us).  Copy/Square are set-resident fillers.
  * The w-sum over models runs on the TensorEngine in float32r (~12-bit
    mantissa, 1 cycle/col vs 4 for fp32).  Rounding noise is zero-mean and
    contributes <0.1% to the final mean; the tau correction uses the same
    rounded rnorm so normalization stays self-consistent.

Layout: 8 cores x 2048 samples.  Per core, 16 groups of 512 rows of the
host-interleaved xall (row 4*s+m = model m of sample s).  Each group is one
[128, 4, 1000] SBUF tile: partition p = 4*s' + m (s' = s mod 32), free dim =
(tile j, class c).  Per group:
  DMA(1): 2 MB contiguous load.
  ACT: exp -> E (f32r) once over [128, 4000]; ln/exp/square for rnorm [128,4];
       4x lhsT = Z_j * rnorm ([128,128] f32r, Z_j inline block-diag masks).
  DVE: one batched bn_stats [128,8,500] + 4 bn_aggr -> var(e); tau partial
       sums; one batched bn_stats over the W PSUM tile -> var(w); accumulate.
  PE:  8 accumulating f32r matmuls stack w for 4 tiles into one [128, 1000]
       PSUM tile (lhsT zero outside col band 32j..32j+32).
Final: PE ones-matmul folds [128,2] (sum var_w | sum tau) across partitions.
Host: answer = SCALE * 0.5 * C * (sum_w - sum_tau) / B.
"""

import sys

import numpy as np

for _p in ("/opt/trn_rl_repo",):
    if _p not in sys.path:
        sys.path.insert(0, _p)

import concourse.bacc as bacc
import concourse.tile as tile
from concourse import mybir
from concourse import bass_utils

B, C = 16384, 1000
N_CORES = 8
B_LOC = B // N_CORES  # 2048
SPT = 32  # samples per tile (x 4 models = 128 partitions)
GROUP = 4  # tiles per PSUM stack / load group
N_GROUPS = B_LOC // (SPT * GROUP)  # 16
T = 20.0
SCALE = 0.3

F32 = mybir.dt.float32
F32R = mybir.dt.float32r
AF = mybir.ActivationFunctionType


def _build_nc():
    nc = bacc.Bacc("TRN2")
    # xall rows are interleaved (sample-major): row 4*s + m = model m of sample s
    xall = nc.dram_tensor("xall", [B_LOC * 4, C], F32, kind="ExternalInput")
    out = nc.dram_tensor("partial", [1, 1], F32, kind="ExternalOutput")

    # Z[j][p, 32*j + p//4] = 1, zero elsewhere: per-tile block-diagonal
    # scatter masks; lhsT_j = Z_j * rnorm stacks tile j's samples into
    # output partitions 32j..32j+32 of the shared accumulating PSUM tile.
    z_np = np.zeros((GROUP, 128, 128), dtype=np.float32)
    for j in range(GROUP):
        z_np[j, np.arange(128), SPT * j + np.arange(128) // 4] = 1.0
    z_dram = nc.inline_tensor(np.ascontiguousarray(z_np), name="zmasks")

    with tile.TileContext(nc) as tc:
        with (
            tc.tile_pool(name="singles", bufs=1) as singles,
            tc.tile_pool(name="xpool", bufs=4) as xpool,
            tc.tile_pool(name="epool", bufs=3) as epool,
            tc.tile_pool(name="small", bufs=6) as small,
            tc.tile_pool(name="lhspool", bufs=8) as lhspool,
            tc.tile_pool(name="wpsum", bufs=2, space="PSUM") as wpsum,
        ):
            zmasks = singles.tile([128, GROUP, 128], F32)
            nc.gpsimd.dma_start(
                out=zmasks, in_=z_dram.rearrange("j p q -> p j q")
            )
            ones = singles.tile([128, 1], F32)
            nc.vector.memset(ones, 1.0)
            warm = singles.tile([128, 1], F32)
            nc.scalar.activation(warm, ones, AF.Exp)
            # accs[:,0] accumulates var(w); accs[:,1] accumulates tau
            accs = singles.tile([128, 2], F32)
            nc.vector.memset(accs, 0.0)

            for g in range(N_GROUPS):
                r0 = g * GROUP * 128  # row offset into xall
                X = xpool.tile([128, GROUP, C], F32, tag="X")
                for j in range(GROUP):
                    nc.sync.dma_start(
                        out=X[:, j, :],
                        in_=xall[r0 + 128 * j : r0 + 128 * (j + 1), :],
                    )
                fine = g == N_GROUPS - 1
                E = epool.tile([128, GROUP, C], F32R, tag="E")
                if fine:
                    # last group: per-tile exp so each tile's chain drains
                    # without waiting for the whole 4000-wide activation
                    for j in range(GROUP):
                        nc.scalar.activation(
                            E[:, j, :], X[:, j, :], AF.Exp, scale=1.0 / T
                        )
                else:
                    nc.scalar.activation(E, X, AF.Exp, scale=1.0 / T)
                Ef = E.bitcast(F32)
                stats = small.tile([128, 2 * GROUP, 6], F32, tag="stats")
                Ev = Ef.rearrange("p j (h x) -> p (j h) x", h=2)
                for h in range(2 * GROUP):
                    nc.vector.bn_stats(stats[:, h, :], Ev[:, h, :])
                mv = small.tile([128, GROUP, 2], F32, tag="mv")
                for j in range(GROUP):
                    nc.vector.bn_aggr(mv[:, j, :], stats[:, 2 * j : 2 * j + 2, :])
                varv = mv[:, :, 1]  # [128, GROUP] strided view
                # rnorm = exp(-0.5 * ln(C * var)); ln+exp share one table set
                L = small.tile([128, GROUP], F32, tag="L")
                rn = small.tile([128, GROUP], F32R, tag="rn")
                if fine:
                    for j in range(GROUP):
                        nc.scalar.activation(
                            L[:, j : j + 1], varv[:, j : j + 1], AF.Ln,
                            scale=float(C),
                        )
                        nc.scalar.activation(
                            rn[:, j : j + 1], L[:, j : j + 1], AF.Exp,
                            scale=-0.5,
                        )
                else:
                    nc.scalar.activation(L, varv, AF.Ln, scale=float(C))
                    nc.scalar.activation(rn, L, AF.Exp, scale=-0.5)
                rnf = rn.bitcast(F32)
                # tau partials: t = var * rnorm^2; only sum_{p,j} t is needed
                tg = small.tile([128, GROUP], F32, tag="tg")
                nc.vector.tensor_mul(tg, rnf, rnf)
                nc.vector.tensor_mul(tg, tg, varv)
                tsum = small.tile([128, 1], F32, tag="tsum")
                nc.vector.tensor_reduce(
                    tsum, tg, axis=mybir.AxisListType.X, op=mybir.AluOpType.add
                )
                nc.vector.tensor_add(accs[:, 1:2], accs[:, 1:2], tsum)

                # rm = rnorm * mean(e): weights for the w-mean (wbar) matmul
                meanv = mv[:, :, 0]
                rm = small.tile([128, GROUP], F32, tag="rm")
                nc.vector.tensor_mul(rm, rnf, meanv)
                W = wpsum.tile([128, 1024], F32, tag="W")
                Wb = wpsum.tile([128, 8], F32, tag="Wb")
                for j in range(GROUP):
                    lhsT = lhspool.tile([128, 128], F32R, tag="lhsT")
                    with nc.allow_low_precision(reason="f32r lhsT for PE"):
                        nc.gpsimd.tensor_scalar_mul(
                            lhsT, zmasks[:, j, :], rnf[:, j : j + 1]
                        )
                    lhsT2 = lhspool.tile([128, 128], F32, tag="lhsT2")
                    nc.gpsimd.tensor_scalar_mul(
                        lhsT2, zmasks[:, j, :], rm[:, j : j + 1]
                    )
                    first, last = (j == 0), (j == GROUP - 1)
                    nc.tensor.matmul(
                        W[:, 0:512],
                        lhsT,
                        E[:, j, 0:512],
                        start=first,
                        stop=last,
                        skip_group_check=True,
                    )
                    nc.tensor.matmul(
                        W[:, 512:1000],
                        lhsT,
                        E[:, j, 512:1000],
                        start=first,
                        stop=last,
                        skip_group_check=True,
                    )
                    nc.tensor.matmul(
                        Wb[:, 0:1],
                        lhsT2,
                        ones,
                        start=first,
                        stop=last,
                        skip_group_check=True,
                    )
                # C*var(w) = sum_c (w - wbar)^2 via ACT Square+accum (both
                # chunks), freeing the DVE of the W-variance stream.
                wbneg = small.tile([128, 1], F32, tag="wbneg")
                nc.scalar.activation(wbneg, Wb[:, 0:1], AF.Copy, scale=-1.0)
                sqscr = small.tile([128, 1024], F32, tag="sqscr", bufs=2)
                q = small.tile([128, 2], F32, tag="q")
                nc.scalar.activation(
                    sqscr[:, 0:512],
                    W[:, 0:512],
                    AF.Square,
                    bias=wbneg,
                    accum_out=q[:, 0:1],
                )
                nc.scalar.activation(
                    sqscr[:, 512:1000],
                    W[:, 512:1000],
                    AF.Square,
                    bias=wbneg,
                    accum_out=q[:, 1:2],
                )
                nc.vector.tensor_add(accs[:, 0:1], accs[:, 0:1], q[:, 0:1])
                nc.vector.tensor_add(accs[:, 0:1], accs[:, 0:1], q[:, 1:2])

            # d-accumulator = accs[:,0] - C*accs[:,1], folded before the
            # cross-partition reduce so the big tau/var magnitudes cancel in
            # f32 while still small.
            dacc = singles.tile([128, 1], F32)
            nc.vector.tensor_scalar(
                dacc,
                accs[:, 1:2],
                float(-C),
                None,
                op0=mybir.AluOpType.mult,
            )
            nc.vector.tensor_add(dacc, dacc, accs[:, 0:1])
            fin = wpsum.tile([1, 1], F32, tag="fin", bufs=1)
            nc.tensor.matmul(fin, dacc, ones, start=True, stop=True)
            res = singles.tile([1, 1], F32)
            nc.scalar.copy(res, fin)
            nc.sync.dma_start(out=out[:, :], in_=res)
    _strip_redundant_dma_waits(nc)
    # Force the ACT table chooser onto the one set that serves every function
    # this kernel uses (exp, ln, square, copy, identity).  The default greedy
    # chooser picks exp_and_others for Exp and natural_log for Ln, inserting
    # a ~2.7us table swap per activation pair, ~86us/core of pure overhead.
    _orig_tables = bacc.get_activation_tables

    def _only_shared(arch):
        tabs = _orig_tables(arch)
        return {
            name: (fns if name == "natural_log_exp_and_others" else set())
            for name, fns in tabs.items()
        }

    bacc.get_activation_tables = _only_shared
    try:
        nc.finalize()
    finally:
        bacc.get_activation_tables = _orig_tables
    return nc


def _strip_redundant_dma_waits(nc):
    """Drop same-queue WAW waits on load DMAs.

    The HWDGE DMA pseudo-instruction accepts a single sync wait, but Tile
    emits two once SBUF slots recycle: [engine-sem release by the slot's
    reader, own-queue WAW vs the slot's previous writer].  The WAW wait is
    redundant when the kept reader wait transitively orders the new write
    after the old one.  Only remove a wait that targets the exact semaphore
    this DMA updates (same queue) while an engine-semaphore wait remains;
    anything else is left for Bacc's event-semaphore splitting.
    """
    removed = 0
    for inst in nc.inst_map.values():
        if type(inst).__name__ != "InstDMACopy":
            continue
        si = getattr(inst, "sync_info", None)
        if si is None:
            continue
        waits = list(si.on_wait or [])
        if len(waits) <= 1:
            continue
        upd_names = {u.ant_name for u in (si.on_update or [])}
        drop = [w for w in waits if w.ant_name in upd_names]
        keep = [w for w in waits if w.ant_name not in upd_names]
        if len(drop) != 1 or not keep:
            continue
        if any(k.ant_name.startswith("DMA") for k in keep):
            continue
        si.on_wait = keep
        inst.sync_info = si
        removed += 1
    return removed


_NC_CACHE = {}


def _get_nc():
    if "nc" not in _NC_CACHE:
        _NC_CACHE["nc"] = _build_nc()
    return _NC_CACHE["nc"]


def run_on_cores(arrays, trace=False):
    """arrays: list of 4 full [B, C] f32 arrays. Returns (partials, results)."""
    nc = _get_nc()
    # Interleave models per sample: xall[4*s + m, c] = arrays[m][s, c]
    xall = np.ascontiguousarray(
        np.stack(arrays, axis=1).reshape(B * 4, C).astype(np.float32, copy=False)
    )
    in_maps = []
    for k in range(N_CORES):
        in_maps.append({"xall": xall[k * B_LOC * 4 : (k + 1) * B_LOC * 4]})
    res = bass_utils.run_bass_kernel_spmd(
        nc, in_maps, core_ids=list(range(N_CORES)), trace=trace
    )
    # per-core partial = sum_b d_b (tau already folded on device)
    partials = [float(r["partial"][0, 0]) for r in res.results]
    return partials, res


def kernel(outputs1, outputs2, outputs3, outputs4, targets=None):
    arrays = [
        np.ascontiguousarray(np.asarray(a, dtype=np.float32))
        for a in (outputs1, outputs2, outputs3, outputs4)
    ]
    partials, _ = run_on_cores(arrays, trace=False)
    total = float(np.sum(np.asarray(partials, dtype=np.float64)))
    ans = SCALE * 0.5 * total / B
    return np.array(ans, dtype=np.float32)
